# revision 25
# baseline (speedup 1.0000x reference)
"""Trainium2 Bass kernel for nn_DattaBotModel (pre-norm causal attention +
top-2-of-8 MoE FFN), 8 NeuronCores.

v2 design:
- Head-parallel attention (core c owns heads 2c,2c+1); attention outputs are
  AllGathered in bf16 (2 token-chunks, overlapped), then every core computes
  the full h = x + ao @ wo.T locally (wo replicated in bf16).
- Dense expert-parallel MoE: core c evaluates expert c over all tokens with
  fp8(e4m3) DoubleRow matmuls (weights pre-scaled x64 on host, rescaled in
  the gelu / combine steps), weights streamed once from HBM.
- Gate logits computed exactly in fp32 from h with moe_norm_w folded into
  gate_w host-side; the per-token rsqrt scale folds into the top-2 softmax
  via a transposed rsqrt row (selection is scale-invariant).
- Output: each core's weighted expert output + h/8 is ReduceScattered (fp32,
  2 token-chunks) so the sum reconstructs h + moe exactly.
"""

import numpy as np
from contextlib import ExitStack

import concourse.bass as bass
import concourse.mybir as mybir
import concourse.tile as tile
from concourse.bass_utils import run_bass_kernel_spmd

F32 = mybir.dt.float32
F32R = mybir.dt.float32r
BF16 = mybir.dt.bfloat16
FP8 = mybir.dt.float8e4
AF = mybir.ActivationFunctionType
OP = mybir.AluOpType
PM = mybir.MatmulPerfMode

P = 128
B, S, D = 2, 1024, 1024
NH, HD = 16, 64
E, H = 8, 4096
T = B * S            # 2048 tokens
NCORES = 8
DT = D // P          # 8 feature tiles
HT = H // P          # 32 hidden tiles
NTB = T // 512       # 4 token blocks of 512
NTI = T // P         # 16 token tiles of 128
HF = T // 2          # half-size (1024 tokens)
EPS = 1e-6
WSCALE = 1.0         # (bf16 FFN: no pre-scale)

MAX_WAITS = 1  # this walrus build rejects >1 sync-wait on one instruction


def _split_waits(nc, limit=MAX_WAITS):
    """Move excess semaphore waits onto standalone NoOps before the owning
    instruction (same engine; waits are ge-conditions so order is free)."""
    n = 0
    for f in nc.m.functions:
        for b in f.blocks:
            out = []
            for inst in b.instructions:
                si = inst.sync_info
                if si is not None and si.on_wait and len(si.on_wait) > limit:
                    waits = list(si.on_wait)
                    sem = [w for w in waits if w.sync_type == "semaphore"]
                    other = [w for w in waits if w.sync_type != "semaphore"]
                    keep = limit - len(other)
                    assert keep >= 1
                    extra, kept = sem[:-keep], sem[-keep:]
                    for i in range(0, len(extra), limit):
                        nop = mybir.InstNoOp(
                            name=f"{inst.name}-wsplit{i}", ins=[], outs=[]
                        )
                        nop.engine = inst.engine
                        nop.sync_info = mybir.SyncInfo(
                            on_wait=list(extra[i : i + limit]), on_update=[]
                        )
                        out.append(nop)
                        n += 1
                    si.on_wait = other + kept
                out.append(inst)
            b.instructions = out
    return n


def r32(ap):
    return ap.bitcast(F32R)


class DmaMux:
    "Round-robin dma_start issue across engines to parallelize DGE issue."
    def __init__(self, nc, engines=None):
        self.engines = engines or [nc.sync, nc.gpsimd, nc.scalar]
        self.i = 0

    def __call__(self, out, in_):
        e = self.engines[self.i % len(self.engines)]
        self.i += 1
        return e.dma_start(out=out, in_=in_)


def build_bass():
    nc = bass.Bass()
    dp = nc.declare_dram_parameter

    xT = dp("xT", [D, T], F32, isOutput=False)               # x transposed
    wqm = dp("wqm", [P, DT, P], BF16, isOutput=False)        # my-heads Q lhsT tiles
    wkm = dp("wkm", [P, DT, P], BF16, isOutput=False)
    wvm = dp("wvm", [P, DT, P], BF16, isOutput=False)
    wot = dp("wot", [P, DT, DT, P], BF16, isOutput=False)    # full wo lhsT [r][dt]
    gwT = dp("gwT", [P, DT, E], F32, isOutput=False)         # (gate_w*nw).T tiles
    w1r = dp("w1r", [HT, P, DT, P], BF16, isOutput=False)            # fc1 lhsT tiles
    w2r = dp("w2r", [DT, P, HT, P], BF16, isOutput=False)            # fc2 lhsT tiles
    b1m = dp("b1m", [P, HT], F32, isOutput=False)
    b2m = dp("b2m", [P, DT], F32, isOutput=False)            # 64*b2 tiles
    nwa = dp("nwa", [1, D], F32R, isOutput=False)             # attn_norm_w row
    cosT = dp("cosT", [P, T], F32, isOutput=False)
    sinT = dp("sinT", [P, T], F32, isOutput=False)           # sign-folded
    mskd = dp("mskd", [P, P], F32, isOutput=False)           # k<=q 0/1
    ident = dp("ident", [P, P], F32R, isOutput=False)
    onesr = dp("onesr", [1, P], F32R, isOutput=False)         # row of ones
    onesc = dp("onesc", [P, 1], F32R, isOutput=False)         # col of ones
    sel = dp("sel", [P, E], F32, isOutput=False)             # one-hot(my expert)/64
    outp = dp("outp", [P, T], F32, isOutput=True)            # my 128 rows of outT

    ag_in = nc.dram_tensor("ag_in", [2, P, HF], BF16)
    ag_out = [
        nc.dram_tensor(f"ag_out{h}", [NCORES, P, HF], BF16, addr_space="Shared")
        for h in range(2)
    ]
    moe_dram = nc.dram_tensor("moe_dram", [2, D, HF], F32)
    scr_r = nc.dram_tensor("scr_r", [T], F32R)
    scr_m = nc.dram_tensor("scr_m", [T], F32R)
    rs_out = nc.dram_tensor("rs_out", [2, P, HF], F32)

    groups = [list(range(NCORES))]
    dma = DmaMux(nc)

    tc = tile.TileContext(nc)
    tc.__enter__()
    ctx = ExitStack()
    ctx.enter_context(nc.allow_low_precision(
        reason="f32r stat rows feed f32r matmuls; error budget accounted"))
    cpool = ctx.enter_context(tc.tile_pool(name="consts", bufs=1))

    # ---- persistent constants ----
    b1_sb = cpool.tile([P, HT], F32, tag="b1")
    dma(out=b1_sb[:], in_=b1m[:])
    b2_sb = cpool.tile([P, DT], F32, tag="b2")
    dma(out=b2_sb[:], in_=b2m[:])
    or_sb = cpool.tile([1, P], F32R, tag="or")
    dma(out=or_sb[:], in_=onesr[:])
    oc_sb = cpool.tile([P, 1], F32R, tag="oc")
    dma(out=oc_sb[:], in_=onesc[:])
    sel_sb = cpool.tile([P, E], F32, tag="sel")
    dma(out=sel_sb[:], in_=sel[:])
    eps_sb = cpool.tile([1, 1], F32, tag="eps")
    nc.vector.memset(eps_sb[:], EPS)
    zc_sb = cpool.tile([P, 1], F32, tag="zc")
    nc.vector.memset(zc_sb[:], 0.0)
    id_sb = cpool.tile([P, P], F32R, tag="id")
    dma(out=id_sb[:], in_=ident[:])
    gw_sb = cpool.tile([P, DT, E], F32, tag="gw")
    dma(out=gw_sb[:], in_=gwT[:])

    qkv_ctx = ExitStack()
    qkv_pool = qkv_ctx.enter_context(tc.tile_pool(name="qkv", bufs=1))
    qT = qkv_pool.tile([P, T], F32R, tag="qT")
    kT = qkv_pool.tile([P, T], F32R, tag="kT")
    v_sb = qkv_pool.tile([P, NTI, 130], F32R, tag="v")
    cos_sb = qkv_pool.tile([P, T], F32, tag="cos")
    dma(out=cos_sb[:], in_=cosT[:])
    sin_sb = qkv_pool.tile([P, T], F32, tag="sin")
    dma(out=sin_sb[:], in_=sinT[:])
    msk_sb = qkv_pool.tile([P, P], F32, tag="msk")
    dma(out=msk_sb[:], in_=mskd[:])
    ao_ctx = ExitStack()
    ao_pool = ao_ctx.enter_context(tc.tile_pool(name="ao", bufs=1))
    aoT = ao_pool.tile([P, T], BF16, tag="aoT")

    # ========== stage 1: t = rmsnorm(x) -> bf16, feature-major ==========
    t_ctx = ExitStack()
    tpool = t_ctx.enter_context(tc.tile_pool(name="tT", bufs=1))
    tT = [tpool.tile([P, T], BF16, tag=f"t{dt}", name=f"t{dt}") for dt in range(DT)]
    wq_sb = tpool.tile([P, DT, P], BF16, tag="wq")
    dma(out=wq_sb[:], in_=wqm[:])
    wk_sb = tpool.tile([P, DT, P], BF16, tag="wk")
    dma(out=wk_sb[:], in_=wkm[:])
    wv_sb = tpool.tile([P, DT, P], BF16, tag="wv")
    dma(out=wv_sb[:], in_=wvm[:])
    nwa_sb = tpool.tile([1, D], F32R, tag="nwa")
    dma(out=nwa_sb[:], in_=nwa[:])
    with tc.tile_pool(name="s1", bufs=2) as s1, \
         tc.tile_pool(name="s1r", bufs=1) as s1r, \
         tc.tile_pool(name="ps1", bufs=1, space="PSUM") as ps1, \
         tc.tile_pool(name="ps1b", bufs=2, space="PSUM") as ps1b:
        ssq = [ps1.tile([1, 512], F32, tag=f"ssq{tb}", name=f"ssq{tb}") for tb in range(NTB)]
        for dt in range(DT):
            xt = s1.tile([P, T], F32, tag="xt")
            dma(out=xt[:], in_=xT[dt * P : (dt + 1) * P, :])
            sq = s1.tile([P, T], F32R, tag="sq")
            nc.vector.tensor_mul(out=sq[:], in0=xt[:], in1=xt[:])
            for tb in range(NTB):
                nc.tensor.matmul(
                    ssq[tb][:], lhsT=oc_sb[:], rhs=sq[:, tb * 512 : (tb + 1) * 512],
                    start=(dt == 0), stop=(dt == DT - 1),
                )
        r_row = s1r.tile([1, T], F32R, tag="rrow")
        for tb in range(NTB):
            srt = s1r.tile([1, 512], F32, tag="srt")
            nc.scalar.activation(
                out=srt[:], in_=ssq[tb][:], func=AF.Sqrt,
                scale=1.0 / D, bias=eps_sb[:],
            )
            nc.vector.reciprocal(
                out=r_row[0:1, tb * 512 : (tb + 1) * 512], in_=srt[:]
            )
        for dt in range(DT):
            xt = s1.tile([P, T], F32, tag="xt")
            dma(out=xt[:], in_=xT[dt * P : (dt + 1) * P, :])
            for tb in range(NTB):
                cs = slice(tb * 512, (tb + 1) * 512)
                rb = ps1b.tile([P, 512], F32, tag="rb")
                nc.tensor.matmul(
                    rb[:], lhsT=nwa_sb[0:1, dt * P : (dt + 1) * P],
                    rhs=r_row[0:1, cs], start=True, stop=True,
                )
                nc.vector.tensor_mul(
                    out=tT[dt][:, cs], in0=xt[:, cs], in1=rb[:]
                )

    # ========== stage 2: QKV (+RoPE on q,k), v via transpose ==========
    with tc.tile_pool(name="ps2", bufs=2, space="PSUM") as ps2, \
         tc.tile_pool(name="ps2t", bufs=2, space="PSUM") as ps2t, \
         tc.tile_pool(name="s2", bufs=1) as s2:
        vT = s2.tile([P, T], F32R, tag="vT")
        for dst, w in ((qT, wq_sb), (kT, wk_sb), (vT, wv_sb)):
            for tb in range(NTB):
                cs = slice(tb * 512, (tb + 1) * 512)
                pp = ps2.tile([P, 512], F32, tag="qk")
                for dt in range(DT):
                    nc.tensor.matmul(
                        pp[:], lhsT=(w[:, dt, :]), rhs=(tT[dt][:, cs]),
                        start=(dt == 0), stop=(dt == DT - 1),
                    )
                nc.scalar.copy(out=dst[:, cs], in_=pp[:])
        # v token-major via PE transpose of vT
        nc.vector.tensor_copy(out=v_sb[:, :, 64], in_=oc_sb[:].to_broadcast([P, NTI]))
        nc.vector.tensor_copy(out=v_sb[:, :, 129], in_=oc_sb[:].to_broadcast([P, NTI]))
        for ti in range(NTI):
            rs = slice(ti * P, (ti + 1) * P)
            tp = ps2t.tile([P, P], F32R, tag="vt")
            nc.tensor.transpose(out=tp[:], in_=vT[:, rs], identity=id_sb[:])
            nc.vector.tensor_copy(out=v_sb[:, ti, 0:64], in_=tp[:, 0:64])
            nc.vector.tensor_copy(out=v_sb[:, ti, 65:129], in_=tp[:, 64:128])
        # RoPE: z' = z*cos + rot(z)*sin_signed
        for z in (qT, kT):
            rot = s2.tile([P, T], F32, tag="rot")
            for hh in range(2):
                o = hh * 64
                nc.vector.tensor_copy(out=rot[o : o + 32, :], in_=z[o + 32 : o + 64, :])
                nc.vector.tensor_copy(out=rot[o + 32 : o + 64, :], in_=z[o : o + 32, :])
            zc = s2.tile([P, T], F32, tag="zcp")
            nc.vector.tensor_mul(out=zc[:], in0=z[:], in1=cos_sb[:])
            nc.vector.tensor_mul(out=rot[:], in0=rot[:], in1=sin_sb[:])
            nc.vector.tensor_add(out=z[:], in0=zc[:], in1=rot[:])
    t_ctx.close()

    # ========== stage 3: attention + chunked AllGather ==========
    with tc.tile_pool(name="ps3", bufs=2, space="PSUM") as ps3, \
         tc.tile_pool(name="ps3a", bufs=2, space="PSUM") as ps3a, \
         tc.tile_pool(name="ps3b", bufs=1, space="PSUM") as ps3b, \
         tc.tile_pool(name="s3", bufs=3) as s3, \
         tc.tile_pool(name="s3b", bufs=2) as s3b:
        for b in range(B):
            for hh in range(2):
                hr = slice(hh * 64, (hh + 1) * 64)
                hv = slice(hh * 65, (hh + 1) * 65)
                aops = []
                for qb in range(2):
                    tb = 2 * b + qb
                    qcs = slice(tb * 512, (tb + 1) * 512)
                    ao = ps3a.tile([65, 512], F32, tag=f"ao{qb}")
                    nkt = 4 * (qb + 1)
                    for kt in range(nkt):
                        off = max(0, (kt - 4 * qb) * P)
                        gkt = b * 8 + kt
                        krs = slice(gkt * P, (gkt + 1) * P)
                        st = ps3.tile([P, 512], F32, tag="st")
                        nc.tensor.matmul(
                            st[:, off:512], lhsT=(kT[hr, krs]),
                            rhs=(qT[hr, tb * 512 + off : (tb + 1) * 512]),
                            start=True, stop=True,
                        )
                        ex = s3.tile([P, 512], F32R, tag="ex")
                        if off:
                            nc.vector.tensor_copy(
                                out=ex[:, 0:off],
                                in_=zc_sb[:].to_broadcast([P, off]),
                            )
                        nc.scalar.activation(
                            out=ex[:, off:512], in_=st[:, off:512],
                            func=AF.Exp, scale=0.125,
                        )
                        if kt >= 4 * qb:
                            nc.vector.tensor_mul(
                                out=ex[:, off : off + P],
                                in0=ex[:, off : off + P], in1=msk_sb[:],
                            )
                        nc.tensor.matmul(
                            ao[:], lhsT=(v_sb[:, gkt, hv]), rhs=(ex[:]),
                            start=(kt == 0), stop=(kt == nkt - 1),
                        )
                    aops.append((ao, qcs))
                for qb, (ao, qcs) in enumerate(aops):
                    rs1 = s3b.tile([1, 512], F32, tag="rs1")
                    nc.scalar.copy(out=rs1[:], in_=ao[64:65, :])
                    rc1 = s3b.tile([1, 512], F32R, tag="rc1")
                    nc.vector.reciprocal(out=rc1[:], in_=rs1[:])
                    nb = ps3b.tile([64, 512], F32, tag="nb")
                    nc.tensor.matmul(
                        nb[:], lhsT=or_sb[0:1, 0:64], rhs=rc1[:],
                        start=True, stop=True,
                    )
                    nbs = s3b.tile([64, 512], F32, tag="nbs")
                    nc.scalar.copy(out=nbs[:], in_=nb[:])
                    nc.vector.tensor_mul(out=aoT[hr, qcs], in0=ao[0:64, :], in1=nbs[:])
            # fire AllGather for this batch's tokens
            hcs = slice(b * HF, (b + 1) * HF)
            dma(out=ag_in[b], in_=aoT[:, hcs])
            nc.gpsimd.collective_compute(
                "AllGather", OP.bypass, replica_groups=groups,
                ins=[ag_in[b]], outs=[ag_out[b][:]],
            )
    ao_ctx.close()
    qkv_ctx.close()

    # ========== stage 4: h = x + wo @ ao (full, per half) + stats ==========
    h_ctx = ExitStack()
    hpool = h_ctx.enter_context(tc.tile_pool(name="h", bufs=1, side="right"))
    h_t = [hpool.tile([P, T], F32, tag=f"h{dt}", name=f"h{dt}") for dt in range(DT)]
    tn8 = hpool.tile([P, DT, T], BF16, tag="tn8")            # 32KB/part
    myw_row = hpool.tile([1, T], F32R, tag="mywrow")
    rows_ctx = ExitStack()
    rows = rows_ctx.enter_context(tc.tile_pool(name="rows", bufs=1))
    r_row = rows.tile([1, T], F32R, tag="rrow5")
    rrt = rows.tile([P, NTI], F32R, tag="rrt")                # token-major rsqrt
    wo_ctx = ExitStack()
    wo_pool = wo_ctx.enter_context(tc.tile_pool(name="wo", bufs=1))
    wo_sb = wo_pool.tile([P, DT, DT, P], BF16, tag="wot")
    dma(out=wo_sb[:], in_=wot[:])
    with tc.tile_pool(name="s4", bufs=2) as s4, \
         tc.tile_pool(name="s4a", bufs=1) as s4a, \
         tc.tile_pool(name="ps4", bufs=3, space="PSUM") as ps4, \
         tc.tile_pool(name="ps4b", bufs=1, space="PSUM") as ps4b:
        ssq = [ps4b.tile([1, 512], F32, tag=f"ssq4{tb}", name=f"ssq4{tb}") for tb in range(NTB)]
        for hf in range(2):
            hcs = slice(hf * HF, (hf + 1) * HF)
            ao_r = []
            for r in range(NCORES):
                a_ = s4a.tile([P, HF], BF16, tag=f"agr{r}")
                dma(out=a_[:], in_=ag_out[hf][r])
                ao_r.append(a_)
            for dt in range(DT):
                xt = s4.tile([P, HF], F32, tag="xt4")
                dma(out=xt[:], in_=xT[dt * P : (dt + 1) * P, hf * HF : (hf + 1) * HF])
                for nb_ in range(2):
                    tb = hf * 2 + nb_
                    cs = slice(tb * 512, (tb + 1) * 512)
                    lcs = slice(nb_ * 512, (nb_ + 1) * 512)
                    pp = ps4.tile([P, 512], F32, tag="h")
                    for r in range(NCORES):
                        nc.tensor.matmul(
                            pp[:], lhsT=(wo_sb[:, r, dt, :]), rhs=(ao_r[r][:, lcs]),
                            start=(r == 0), stop=(r == NCORES - 1),
                        )
                    nc.vector.tensor_add(out=h_t[dt][:, cs], in0=pp[:], in1=xt[:, lcs])
                    sq = s4.tile([P, 512], F32R, tag="sq4")
                    nc.vector.tensor_mul(
                        out=sq[:], in0=h_t[dt][:, cs], in1=h_t[dt][:, cs]
                    )
                    nc.tensor.matmul(
                        ssq[tb][:], lhsT=oc_sb[:], rhs=sq[:],
                        start=(dt == 0), stop=(dt == DT - 1),
                    )
        # r_row = 1/sqrt(mean+eps); token-major copy via DMA transpose
        for tb in range(NTB):
            srt = s4.tile([1, 512], F32, tag="srt5")
            nc.scalar.activation(
                out=srt[:], in_=ssq[tb][:], func=AF.Sqrt,
                scale=1.0 / D, bias=eps_sb[:],
            )
            nc.vector.reciprocal(
                out=r_row[0:1, tb * 512 : (tb + 1) * 512], in_=srt[:]
            )
        dma(out=scr_r[:], in_=r_row[:])
        dma(out=rrt[:], in_=scr_r[:].rearrange("(t p) -> p t", p=P))
    wo_ctx.close()

    # ========== stage 5: rmsnorm scale, tn8, gate logits + routing ==========
    with tc.tile_pool(name="s5", bufs=2) as s5, \
         tc.tile_pool(name="s5r", bufs=1) as s5r, \
         tc.tile_pool(name="ps5", bufs=2, space="PSUM") as ps5, \
         tc.tile_pool(name="ps5c", bufs=1, space="PSUM") as ps5c:
        # tn8 = h * rsqrt (moe_norm_w folded into fc1/gate weights host-side)
        for dt in range(DT):
            for tb in range(NTB):
                cs = slice(tb * 512, (tb + 1) * 512)
                rb = ps5.tile([P, 512], F32, tag="rb5")
                nc.tensor.matmul(
                    rb[:], lhsT=or_sb[:],
                    rhs=r_row[0:1, cs], start=True, stop=True,
                )
                nc.vector.tensor_mul(out=tn8[:, dt, cs], in0=h_t[dt][:, cs], in1=rb[:])
        # gate logits (exact fp32 from h; gw has moe_norm_w folded in)
        log_ps = ps5c.tile([P, NTI * E], F32, tag="log")
        for ti in range(NTI):
            for dt in range(DT):
                nc.tensor.matmul(
                    log_ps[:, ti * E : (ti + 1) * E],
                    lhsT=h_t[dt][:, ti * P : (ti + 1) * P],
                    rhs=gw_sb[:, dt, :],
                    start=(dt == 0), stop=(dt == DT - 1),
                )
        log_sb = s5r.tile([P, NTI, E], F32, tag="log")
        nc.scalar.copy(
            out=log_sb[:].rearrange("p a b -> p (a b)"), in_=log_ps[:]
        )
        srt8 = s5r.tile([P, NTI, E], F32, tag="srt8")
        for ti in range(NTI):
            nc.vector.max(out=srt8[:, ti], in_=log_sb[:, ti])
        m1 = srt8[:, :, 0]
        m2 = srt8[:, :, 1]
        dm = s5r.tile([P, NTI], F32, tag="dm")
        nc.vector.tensor_sub(out=dm[:], in0=m2, in1=m1)
        # fold per-token rsqrt into the top-2 softmax gap
        nc.vector.tensor_mul(out=dm[:], in0=dm[:], in1=rrt[:])
        exr = s5r.tile([P, NTI], F32, tag="exr")
        nc.scalar.activation(out=exr[:], in_=dm[:], func=AF.Exp)
        den = s5r.tile([P, NTI], F32, tag="den")
        nc.vector.tensor_scalar_add(den[:], exr[:], 1.0)
        p1 = s5r.tile([P, NTI], F32, tag="p1")
        nc.vector.reciprocal(out=p1[:], in_=den[:])
        p2 = s5r.tile([P, NTI], F32, tag="p2")
        nc.vector.tensor_scalar(
            out=p2[:], in0=p1[:], scalar1=-1.0, scalar2=-1.0,
            op0=OP.mult, op1=OP.subtract,
        )
        wsum = s5r.tile([P, NTI, E], F32, tag="wsum")
        mk = s5r.tile([P, NTI, E], F32, tag="mk")
        nc.vector.tensor_tensor(
            out=mk[:], in0=log_sb[:],
            in1=srt8[:, :, 0:1].to_broadcast([P, NTI, E]), op=OP.is_equal,
        )
        nc.vector.tensor_tensor(
            out=wsum[:], in0=mk[:],
            in1=p1[:].unsqueeze(2).to_broadcast([P, NTI, E]), op=OP.mult,
        )
        nc.vector.tensor_tensor(
            out=mk[:], in0=log_sb[:],
            in1=srt8[:, :, 1:2].to_broadcast([P, NTI, E]), op=OP.is_equal,
        )
        nc.vector.scalar_tensor_tensor(
            out=mk[:], in0=mk[:], scalar=1.0,
            in1=p2[:].unsqueeze(2).to_broadcast([P, NTI, E]),
            op0=OP.mult, op1=OP.mult,
        )
        nc.vector.tensor_add(out=wsum[:], in0=wsum[:], in1=mk[:])
        # my expert's weight per token (scaled 1/64 via sel)
        nc.vector.tensor_tensor(
            out=wsum[:], in0=wsum[:],
            in1=sel_sb[:].unsqueeze(1).to_broadcast([P, NTI, E]), op=OP.mult,
        )
        myw = s5r.tile([P, NTI], F32R, tag="myw")
        nc.vector.reduce_sum(out=myw[:], in_=wsum[:], axis=mybir.AxisListType.X)
        dma(out=scr_m[:].rearrange("(p t) -> p t", t=NTI), in_=myw[:])
        dma(out=myw_row[:], in_=scr_m[:].rearrange("(p t) -> t p", t=NTI))

    rows_ctx.close()

    # ========== stage 6: fp8 DoubleRow FFN per token-half + RS ==========
    with tc.tile_pool(name="s6h", bufs=1) as s6h, \
         tc.tile_pool(name="s6w1", bufs=3) as s6w1, \
         tc.tile_pool(name="s6w2", bufs=2) as s6w2, \
         tc.tile_pool(name="s6o", bufs=2) as s6o, \
         tc.tile_pool(name="s6m", bufs=2) as s6m, \
         tc.tile_pool(name="ps6a", bufs=3, space="PSUM") as ps6a, \
         tc.tile_pool(name="ps6b", bufs=3, space="PSUM") as ps6b, \
         tc.tile_pool(name="ps6c", bufs=2, space="PSUM") as ps6c:
        hid8 = s6h.tile([P, HT, HF], BF16, tag="hid8")
        for hf in range(2):
            # fc1 for this half
            for ht in range(HT):
                w1_sb = s6w1.tile([P, DT, P], BF16, tag="w1")
                dma(out=w1_sb[:], in_=w1r[ht])
                for nb_ in range(2):
                    tb = hf * 2 + nb_
                    cs = slice(tb * 512, (tb + 1) * 512)
                    lcs = slice(nb_ * 512, (nb_ + 1) * 512)
                    hp = ps6a.tile([P, 512], F32, tag="h6")
                    for dt in range(DT):
                        nc.tensor.matmul(
                            hp[:], lhsT=(w1_sb[:, dt, :]),
                            rhs=(tn8[:, dt, cs]),
                            start=(dt == 0), stop=(dt == DT - 1),
                        )
                    nc.scalar.activation(
                        out=hid8[:, ht, lcs], in_=hp[:],
                        func=AF.Gelu, bias=b1_sb[:, ht : ht + 1],
                    )
            # fc2 for this half: w2 streamed once per dot, both token blocks
            wb_sb = s6m.tile([P, HF], F32, tag="wbs")
            for nb_ in range(2):
                tb = hf * 2 + nb_
                wb_ps = ps6c.tile([P, 512], F32, tag="wb")
                for j in range(4):
                    ti = tb * 4 + j
                    nc.tensor.matmul(
                        wb_ps[:, j * P : (j + 1) * P], lhsT=or_sb[:],
                        rhs=myw_row[0:1, ti * P : (ti + 1) * P],
                        start=True, stop=True, skip_group_check=True,
                    )
                nc.scalar.copy(
                    out=wb_sb[:, nb_ * 512 : (nb_ + 1) * 512], in_=wb_ps[:]
                )
            for dot in range(DT):
                w2_sb = s6w2.tile([P, HT, P], BF16, tag="w2")
                dma(out=w2_sb[:], in_=w2r[dot])
                for nb_ in range(2):
                    tb = hf * 2 + nb_
                    cs = slice(tb * 512, (tb + 1) * 512)
                    lcs = slice(nb_ * 512, (nb_ + 1) * 512)
                    ep = ps6b.tile([P, 512], F32, tag="e6")
                    for ht in range(HT):
                        nc.tensor.matmul(
                            ep[:], lhsT=(w2_sb[:, ht, :]),
                            rhs=(hid8[:, ht, lcs]),
                            start=(ht == 0), stop=(ht == HT - 1),
                        )
                    mo = s6o.tile([P, 512], F32, tag="mo")
                    # (ep + b2) * rw
                    nc.vector.scalar_tensor_tensor(
                        out=mo[:], in0=ep[:], scalar=b2_sb[:, dot : dot + 1],
                        in1=wb_sb[:, lcs], op0=OP.add, op1=OP.mult,
                    )
                    # + h/8 so the ReduceScatter sum reconstructs h exactly
                    nc.vector.scalar_tensor_tensor(
                        out=mo[:], in0=h_t[dot][:, cs], scalar=0.125,
                        in1=mo[:], op0=OP.mult, op1=OP.add,
                    )
                    dma(
                        out=moe_dram[hf, dot * P : (dot + 1) * P,
                                     nb_ * 512 : (nb_ + 1) * 512],
                        in_=mo[:],
                    )
            nc.gpsimd.collective_compute(
                "ReduceScatter", OP.add, replica_groups=groups,
                ins=[moe_dram[hf]], outs=[rs_out[hf]],
            )
            dma(out=outp[:, hf * HF : (hf + 1) * HF], in_=rs_out[hf])

    h_ctx.close()
    ctx.close()
    tc.__exit__(None, None, None)
    return nc


def host_inputs(x, attn_norm_w, wq, wk, wv, wo, moe_norm_w, gate_w, w1, b1, w2, b2):
    """Per-core input maps (shared arrays referenced, per-core weight shards)."""
    import ml_dtypes
    f = np.float32
    f8 = ml_dtypes.float8_e4m3
    bf = ml_dtypes.bfloat16
    xT = np.ascontiguousarray(x.reshape(T, D).T, dtype=f)
    inv = 1.0 / (10000.0 ** (np.arange(0, HD, 2, dtype=np.float64) / HD))
    fr = np.arange(S, dtype=np.float64)[:, None] * inv
    emb = np.concatenate([fr, fr], -1)                     # [S, 64]
    cos_h = np.cos(emb).T.astype(f)                        # [64, S]
    sin_h = np.sin(emb).T.astype(f)
    sin_sgn = sin_h.copy()
    sin_sgn[0:32] *= -1.0
    cosT = np.tile(np.concatenate([cos_h, cos_h], 0), (1, B))
    sinT = np.tile(np.concatenate([sin_sgn, sin_sgn], 0), (1, B))
    mskd = (np.arange(P)[:, None] <= np.arange(P)[None, :]).astype(f)
    ident = np.eye(P, dtype=f)
    onesr = np.ones((1, P), f)
    onesc = np.ones((P, 1), f)
    nwa = np.ascontiguousarray(attn_norm_w[None, :], dtype=f)
    # fold moe_norm_w into the gate weight (logits computed from h directly)
    gwn = (np.asarray(gate_w, f) * np.asarray(moe_norm_w, f)[None, :])
    gwT = np.ascontiguousarray(gwn.T.reshape(DT, P, E).transpose(1, 0, 2), dtype=f)
    # full wo lhsT tiles: wot[p, r, dt, q] = wo[dt*P+q, r*P+p]
    wot = np.ascontiguousarray(
        wo.T.reshape(DT, P, DT, P).transpose(1, 0, 2, 3), dtype=bf
    )
    nwm = np.asarray(moe_norm_w, f)
    maps = []
    for c in range(NCORES):
        R = slice(P * c, P * (c + 1))
        sel_ = np.zeros((P, E), f)
        sel_[:, c] = 1.0 / WSCALE
        # fc1 weights: fold moe_norm_w (tn8 excludes it), scale x64, fp8
        w1s = (np.asarray(w1[c], f) * nwm[None, :])
        # lhsT[ht, k, dt, m] = w1s[ht*P+m, dt*P+k]
        w1t = w1s.T.reshape(DT, P, HT, P).transpose(2, 1, 0, 3)
        w1r_ = np.ascontiguousarray(w1t, dtype=bf)         # [HT, P, DT, P]
        w2s = np.asarray(w2[c], f)
        # lhsT[dot, k, ht, m] = w2s[dot*P+m, ht*P+k]
        w2t = w2s.T.reshape(HT, P, DT, P).transpose(2, 1, 0, 3)
        w2r_ = np.ascontiguousarray(w2t, dtype=bf)         # [DT, P, HT, P]
        m = {
            "xT": xT, "cosT": cosT, "sinT": sinT, "mskd": mskd, "ident": ident,
            "onesr": onesr, "onesc": onesc, "nwa": nwa, "gwT": gwT,
            "sel": sel_, "wot": wot,
            "wqm": np.ascontiguousarray(
                wq[R, :].T.reshape(DT, P, P).transpose(1, 0, 2), dtype=bf),
            "wkm": np.ascontiguousarray(
                wk[R, :].T.reshape(DT, P, P).transpose(1, 0, 2), dtype=bf),
            "wvm": np.ascontiguousarray(
                wv[R, :].T.reshape(DT, P, P).transpose(1, 0, 2), dtype=bf),
            "w1r": w1r_,
            "w2r": w2r_,
            "b1m": np.ascontiguousarray(b1[c].reshape(HT, P).T, dtype=f),
            "b2m": np.ascontiguousarray(b2[c].reshape(DT, P).T * WSCALE, dtype=f),
        }
        maps.append(m)
    return maps


_CACHE = {}


def kernel(**inputs):
    inputs = {k: np.asarray(v) for k, v in inputs.items()}
    if "nc" not in _CACHE:
        _CACHE["nc"] = build_bass()
        _CACHE["nsplit"] = _split_waits(_CACHE["nc"])
    nc = _CACHE["nc"]
    in_maps = host_inputs(**inputs)
    res = run_bass_kernel_spmd(nc, in_maps, list(range(NCORES)))
    outT = np.concatenate([res.results[c]["outp"] for c in range(NCORES)], 0)
    return np.ascontiguousarray(outT.T).reshape(B, S, D).astype(np.float32)


if __name__ == "__main__":
    rng = np.random.default_rng(0)
    ins = {
        "x": rng.standard_normal((B, S, D), dtype=np.float32),
        "attn_norm_w": np.ones(D, np.float32),
        "wq": rng.standard_normal((D, D), dtype=np.float32) * 0.02,
        "wk": rng.standard_normal((D, D), dtype=np.float32) * 0.02,
        "wv": rng.standard_normal((D, D), dtype=np.float32) * 0.02,
        "wo": rng.standard_normal((D, D), dtype=np.float32) * 0.02,
        "moe_norm_w": np.ones(D, np.float32),
        "gate_w": rng.standard_normal((E, D), dtype=np.float32) * 0.02,
        "w1": rng.standard_normal((E, H, D), dtype=np.float32) * 0.02,
        "b1": np.zeros((E, H), np.float32),
        "w2": rng.standard_normal((E, D, H), dtype=np.float32) * 0.02,
        "b2": np.zeros((E, D), np.float32),
    }
    out = kernel(**ins)
    print(out.shape, out.dtype, np.abs(out).max())
